# revision 37
# baseline (speedup 1.0000x reference)
"""Trainium2 Bass kernel for nn_CAMD_9990093930844 (sparse_attention).
Math: the reference computes, per modality m,
    out_m[i, :] = Q[i] @ S_m(t1[i]) ,  S_m(t) = sum_{j: t2_m[j] <= t} K_m[j] (x) V_m[j]
and returns (sum_m out_m)[:, :2].  Only V[:, :2] matters, so this is
    out[i, v] = sum_m sum_{j: t2_m[j] <= t1[i]} (Q[i] . K_m[j]) * V_m[j, v]
Both t1 and t2_m are sorted, so the rank deviation |p_m[i] - i| (p =
searchsorted) is bounded (~90 for this data).  Each 128-query block b
therefore only needs:
  - an unconditional prefix state over key chunks [0, b-1)
  - a masked local attention over key chunks {b-1, b, b+1}
Sharding: 8 cores = 4 modalities x 2 query halves, host sums the per-core
partial outputs.  Uniform SPMD program; pad chunks carry V=0 / t2=+inf so
they contribute nothing.
Device-program structure (all fp16 except timestamps / PSUM / output):
  - 3-layer MLPs for Q (2048 cols) and K (33 chunk-slots, window chunks
    first in the buffer).  PSUM->SBUF bias(+relu) writebacks are spread
    over Act / DVE / Pool.
  - Prefix-only chunks skip MLP layer 3: csum_c = W3^T (h2_c @ V_c)
    + b3 (sum_j V_cj)^T  (the "G-trick"), using a transpose of h2.
  - Window chunks: scores = kT_c^T qT (fp16, 1 cycle/row), timestamp mask
    fused on DVE/Pool into fp16 msc tiles.
  - All small matmuls are flipped so the moving operand has 2 columns
    (AV: stationary=msc chunk, moving=V; prefix apply: stationary=qT
    block, moving=csum/segment column) and accumulate per-block into one
    [128, 32] PSUM tile -> single copy -> single output DMA (q-major).
"""
import numpy as np
T = 4096
D = 66
M = 4
PC = 128                 # rows per chunk (partition dim)
QCH = 16                 # query blocks per core
KCH = 33                 # local key chunk slots per core
NWIN = 18                # window slots (local chunks 15..32)
NPRE = 15                # prefix-only chunks (local chunks 0..14)
QW = QCH * PC            # 2048 queries per core
KW = KCH * PC            # 4224 local keys per core
TBIG = 6.0e4             # timestamp sentinel for padded keys (> any real t, fp16-finite)
# buffer slot j -> local chunk index
PERM = list(range(15, 33)) + list(range(0, 15))

def _shard_host(x1, x2, x3, x4, wq, bq, wk, bk):
    """Build the 8 per-core input maps (host-side sharding/layout)."""
    xs = [np.asarray(x)[0, 0] for x in (x1, x2, x3, x4)]   # (4096, 66) each
    x1f = xs[0]
    wall = np.concatenate([np.asarray(wq), np.asarray(wk)], 0).astype(np.float32)
    ball = np.concatenate([np.asarray(bq), np.asarray(bk)], 0).astype(np.float32)
    wall_t = np.ascontiguousarray(
        wall.transpose(1, 0, 2).reshape(D, 6 * D)).astype(np.float16)
    ball_t = np.ascontiguousarray(ball.T).astype(np.float32)      # (66, 6)
    in_maps = []
    for core in range(8):
        m, h = core // 2, core % 2
        xm = xs[m]
        xq = np.ascontiguousarray(x1f[2048 * h: 2048 * h + 2048, :].T)
        # Local key chunks: local chunk lk = global chunk lk + 16*(h-1)
        g0 = 16 * (h - 1)
        kbuf = np.zeros((KW, D), np.float32)       # local-chunk order
        vbuf = np.zeros((KW, 2), np.float32)
        tbuf = np.full((KW,), TBIG, np.float32)
        lo_l = max(0, -g0)
        hi_l = min(KCH, 32 - g0)
        gl0 = (lo_l + g0) * PC
        gl1 = (hi_l + g0) * PC
        kbuf[lo_l * PC: hi_l * PC] = xm[gl0:gl1]
        vbuf[lo_l * PC: hi_l * PC] = xm[gl0:gl1, 0:2]
        tbuf[lo_l * PC: hi_l * PC] = xm[gl0:gl1, 65]
        # permute chunks into buffer-slot order (window chunks first)
        kc = kbuf.reshape(KCH, PC, D)[PERM].reshape(KW, D)
        vc = vbuf.reshape(KCH, PC, 2)[PERM]                     # (33,128,2)
        tc = tbuf.reshape(KCH, PC)[PERM]                        # (33,128)
        im = {
            "xq": np.ascontiguousarray(xq).astype(np.float16),
            "xk": np.ascontiguousarray(kc.T).astype(np.float16),
            "v16": np.ascontiguousarray(
                vc.transpose(1, 0, 2).reshape(PC, KCH * 2)).astype(np.float16),
            "t2s": np.ascontiguousarray(tc.T).astype(np.float16),   # (128,33)
            "t1b": np.ascontiguousarray(np.broadcast_to(
                x1f[2048 * h: 2048 * h + 2048, 65][None, :].astype(np.float16),
                (PC, QW))),
            "wall": wall_t,
            "ball": ball_t,
        }
        in_maps.append(im)
    return in_maps

def _window_ok(x1, xs):
    """Check the bounded-rank-deviation assumption the device program needs."""
    t1 = np.asarray(x1)[0, 0, :, 65]
    for xm in xs:
        t2 = np.asarray(xm)[0, 0, :, 65]
        p = np.searchsorted(t2, t1, side="right")
        b = np.arange(32)
        if not (p[b * PC] >= (b - 1) * PC).all():
            return False
        if not (p[b * PC + PC - 1] <= (b + 2) * PC).all():
            return False
    return True

def _nseg(lb):
    return min(3, (lb + 15) // 8)

def _core_emulate(im):
    """Numpy emulation of the device program for one core (validation)."""
    ws = [im["wall"].astype(np.float32)[:, i * D:(i + 1) * D] for i in range(6)]
    bs = [im["ball"].astype(np.float32)[:, i] for i in range(6)]
    def layer(x_T, w, b, relu):
        h = w.T @ x_T + b[:, None]
        return np.maximum(h, 0.0) if relu else h
    xq = im["xq"].astype(np.float32)
    xk = im["xk"].astype(np.float32)
    qT = layer(layer(layer(xq, ws[0], bs[0], 1), ws[1], bs[1], 1),
               ws[2], bs[2], 0)                                  # (66,2048)
    h2 = layer(layer(xk, ws[3], bs[3], 1), ws[4], bs[4], 1)      # (66,4224)
    kT_win = layer(h2[:, :NWIN * PC], ws[5], bs[5], 0)           # (66,2304)
    v = im["v16"].astype(np.float32).reshape(PC, KCH, 2).transpose(1, 0, 2)
    t2 = im["t2s"].astype(np.float32).T                           # (33,128)
    t1 = im["xq"][65].astype(np.float32)                          # (2048,)
    # chunk sums csum[c] for local chunks c = 0..29
    csum = np.zeros((30, D, 2), np.float32)
    for c in range(30):
        if c >= 15:
            w = c - 15                       # window slot
            csum[c] = kT_win[:, w * PC:(w + 1) * PC] @ v[w]
        else:
            j = 18 + c                       # prefix slot
            G = h2[:, j * PC:(j + 1) * PC] @ v[j]                 # (66,2)
            csum[c] = ws[5].T @ G + bs[5][:, None] * v[j].sum(0)[None, :]
    seg = np.stack([csum[0:8].sum(0), csum[8:16].sum(0), csum[16:24].sum(0)])
    out = np.zeros((PC, QCH * 2), np.float32)
    for lb in range(QCH):
        qb = qT[:, lb * PC:(lb + 1) * PC]                        # (66,128)
        acc = np.zeros((PC, 2), np.float32)
        ns = _nseg(lb)
        for s in range(ns):
            acc += qb.T @ seg[s]
        for c in range(8 * ns, lb + 15):
            acc += qb.T @ csum[c]
        for wo in range(3):
            w = lb + wo                      # window slot
            sc = kT_win[:, w * PC:(w + 1) * PC].T @ qb           # (128k,128q)
            msk = (t1[None, lb * PC:(lb + 1) * PC] >=
                   t2[w][:, None]).astype(np.float32)
            acc += (sc * msk).T @ v[w]
        out[:, 2 * lb:2 * lb + 2] = acc
    return out                                                    # (128,32)

def _combine(per_core_outs):
    full = np.zeros((T, 2), np.float32)
    for core, o in enumerate(per_core_outs):
        h = core % 2
        o = np.asarray(o).reshape(PC, QCH, 2)
        full[2048 * h: 2048 * h + 2048] += \
            o.transpose(1, 0, 2).reshape(QW, 2)
    return full[None, :, :]

def _numpy_fallback(x1, x2, x3, x4, wq, bq, wk, bk):
    """Exact dense fallback (used only if the window assumption fails)."""
    xs = [np.asarray(x)[0, 0].astype(np.float64) for x in (x1, x2, x3, x4)]
    def mlp(x, W, b):
        h = x
        for l in range(2):
            h = np.maximum(h @ W[l] + b[l], 0.0)
        return h @ W[2] + b[2]
    Q = mlp(xs[0], np.asarray(wq, np.float64), np.asarray(bq, np.float64))
    t1 = xs[0][:, 65]
    out = np.zeros((T, 2))
    for m in range(M):
        Km = mlp(xs[m], np.asarray(wk, np.float64), np.asarray(bk, np.float64))
        t2 = xs[m][:, 65]
        mask = t2[None, :] <= t1[:, None]
        A = (Q @ Km.T) * mask
        out += A @ xs[m][:, 0:2]
    return out[None].astype(np.float32)

# ---------------------------------------------------------------------------
# Bass device program
# ---------------------------------------------------------------------------
_NC_CACHE = {}

def _build_nc():
    import concourse.bacc as bacc
    import concourse.mybir as mybir
    import concourse.tile as tile
    from concourse import masks
    f32 = mybir.dt.float32
    f16 = mybir.dt.float16
    AF = mybir.ActivationFunctionType
    ALU = mybir.AluOpType
    nc = bacc.Bacc("TRN2", target_bir_lowering=False, debug=False,
                   enable_asserts=False, num_devices=8)
    xq_d = nc.dram_tensor("xq", [D, QW], f16, kind="ExternalInput")
    xk_d = nc.dram_tensor("xk", [D, KW], f16, kind="ExternalInput")
    v16_d = nc.dram_tensor("v16", [PC, KCH * 2], f16, kind="ExternalInput")
    t2s_d = nc.dram_tensor("t2s", [PC, KCH], f16, kind="ExternalInput")
    t1b_d = nc.dram_tensor("t1b", [PC, QW], f16, kind="ExternalInput")
    wall_d = nc.dram_tensor("wall", [D, 6 * D], f16, kind="ExternalInput")
    ball_d = nc.dram_tensor("ball", [D, 6], f32, kind="ExternalInput")
    out_d = nc.dram_tensor("out", [PC, QCH * 2], f32, kind="ExternalOutput")
    # elementwise-engine weights for MLP writebacks (per 15 pieces:
    # Act 10, DVE 3, Pool 2 -- matches engine col/ns rates)
    def wb_eng_for(kind, l, i):
        if l == 2:
            return "a"
        if kind == "k":
            if i >= 6:                      # prefix chunks (late, Act)
                return "a"
            return "d" if i % 2 == 0 else "a"
        return "d" if i in (1, 3) else "a"  # Q l0/l1
    # mask engine per window slot: "d" = DVE STT straight from PSUM,
    # "p" = Act copies scores to SBUF, Pool does the STT (GPSIMD cannot
    # read PSUM on real hardware)
    mask_eng = ["d"] * 18
    with tile.TileContext(nc) as tc:
        with (
            tc.tile_pool(name="const", bufs=1) as cpool,
            tc.tile_pool(name="big", bufs=1) as bpool,
            tc.tile_pool(name="ps_mlp", bufs=4, space="PSUM") as ps_mlp,
            tc.tile_pool(name="ps_sc", bufs=2, space="PSUM") as ps_sc,
            tc.tile_pool(name="ps_tr", bufs=1, space="PSUM") as ps_tr,
            tc.tile_pool(name="ps_sm", bufs=1, space="PSUM") as ps_sm,
        ):
            # ---- tiles
            wsb = cpool.tile([D, 6 * D], f16)
            bsb = cpool.tile([D, 6], f32)
            b3row = cpool.tile([1, D], f16)
            xq = bpool.tile([D, QW], f16)
            xk = bpool.tile([D, KW], f16)
            v16 = bpool.tile([PC, KCH * 2], f16)
            t2s = bpool.tile([PC, KCH], f16)
            t1b = bpool.tile([PC, QW], f16)
            # ---- input DMAs, ordered by first use (t1b is built on-device
            #      by a Pool partition-broadcast of xq row 65)
            nc.sync.dma_start(wsb[:], wall_d[:])
            nc.sync.dma_start(xk[:, 0:384], xk_d[:, 0:384])
            nc.sync.dma_start(xq[:, 0:512], xq_d[:, 0:512])
            nc.sync.dma_start(xk[:, 384:2304], xk_d[:, 384:2304])
            nc.sync.dma_start(xq[:, 512:2048], xq_d[:, 512:2048])
            nc.sync.dma_start(xk[:, 2304:4224], xk_d[:, 2304:4224])
            nc.sync.dma_start(v16[:], v16_d[:])
            nc.sync.dma_start(t2s[:], t2s_d[:])
            nc.sync.dma_start(t1b[:, 0:1024], t1b_d[:, 0:1024])
            nc.sync.dma_start(t1b[:, 1024:2048], t1b_d[:, 1024:2048])
            nc.scalar.dma_start(bsb[:], ball_d[:])
            ident = cpool.tile([128, 128], f32)
            masks.make_identity(nc, ident[:])
            ones128 = cpool.tile([128, 1], f16)
            nc.gpsimd.memset(ones128[:], 1.0)
            # single PSUM bank carved into the small accumulation regions
            smA = ps_sm.tile([128, 512], f32, name="smA")
            outp = smA[:, 0:QCH * 2]
            ssps = smA[:1, 32:92]
            csps = smA[:D, 92:152]
            # ---- PE p-state warmup (outputs unused)
            trA = ps_tr.tile([PC, 2 * 3 * D], f32, name="trA")
            for i in range(12):
                nc.tensor.transpose(trA[:, 0:PC], ident[:], ident[:])
            b3ps = smA[:1, 152:152 + D]
            # ---- MLPs --------------------------------------------------
            hq0 = bpool.tile([D, QW], f16)
            hq1 = bpool.tile([D, QW], f16)
            qT = bpool.tile([D, QW], f16)
            hk0 = bpool.tile([D, KW], f16)
            hk1 = bpool.tile([D, KW], f16)
            kT = bpool.tile([D, NWIN * PC], f16)
            def wb(eng, dst, ps_ap, b_ap, relu):
                if eng == "a":
                    nc.scalar.activation(dst, ps_ap,
                                         AF.Relu if relu else AF.Identity,
                                         bias=b_ap)
                elif eng == "d":
                    if relu:
                        nc.vector.tensor_scalar(dst, ps_ap, b_ap, 0.0,
                                                ALU.add, ALU.max)
                    else:
                        nc.vector.tensor_scalar_add(dst, ps_ap, b_ap)
                else:
                    if relu:
                        nc.gpsimd.tensor_scalar(dst, ps_ap, b_ap, 0.0,
                                                ALU.add, ALU.max)
                    else:
                        nc.gpsimd.tensor_scalar_add(dst, ps_ap, b_ap)
            def mlp_block(src, dst, wofs, c0, cw, relu, wb_eng="a"):
                w_ap = wsb[:, wofs * D:(wofs + 1) * D]
                b_ap = bsb[:, wofs:wofs + 1]
                ps = ps_mlp.tile([D, 512], f32, tag="mlp",
                                 name=f"mlp{wofs}_{c0}")
                nc.tensor.matmul(ps[:, :cw], w_ap, src[:, c0:c0 + cw],
                                 start=True, stop=True)
                eng = wb_eng
                wb(eng, dst[:, c0:c0 + cw], ps[:, :cw], b_ap, relu)
            # K blocks: 11 x 384 (blocks 0..5 = window slots, 6..10 = prefix)
            # Q blocks: 4 x 512
            kb = [(i * 384, 384) for i in range(11)]
            qb = [(i * 512, 512) for i in range(4)]
            # ---- downstream machinery ---------------------------------
            km = bpool.tile([PC, 30 * D], f16)   # key-major K, chunks 0..29
            csb = cpool.tile([D, 60], f16)
            ssb = cpool.tile([1, 60], f16)
            mscs = []
            for w in range(NWIN):
                mscs.append(bpool.tile([PC, 384], f16, name=f"msc{w}"))
            outb = bpool.tile([PC, QCH * 2], f32)
            def score_mask(w):
                lb0 = max(0, w - 2)
                lb1 = min(QCH - 1, w)
                ncol = (lb1 - lb0 + 1) * PC
                ps = ps_sc.tile([PC, 384], f32, tag="sc", name=f"sc{w}")
                nc.tensor.matmul(ps[:, :ncol], kT[:, w * PC:(w + 1) * PC],
                                 qT[:, lb0 * PC:(lb1 + 1) * PC],
                                 start=True, stop=True)
                if mask_eng[w] == "d":
                    nc.vector.scalar_tensor_tensor(
                        mscs[w][:, :ncol],
                        t1b[:, lb0 * PC:(lb1 + 1) * PC],
                        t2s[:, w:w + 1],
                        ps[:, :ncol],
                        ALU.is_ge, ALU.mult)
                else:
                    scb = bpool.tile([PC, 384], f16, name=f"scb{w}")
                    nc.scalar.copy(scb[:, :ncol], ps[:, :ncol])
                    nc.gpsimd.scalar_tensor_tensor(
                        mscs[w][:, :ncol],
                        t1b[:, lb0 * PC:(lb1 + 1) * PC],
                        t2s[:, w:w + 1],
                        scb[:, :ncol],
                        ALU.is_ge, ALU.mult)
            def win_transposes(b):
                # key-major K for csum chunks 15..29 (slots 0..14), 3 per batch
                s0, s1 = 3 * b, min(3 * b + 3, NPRE)
                pst = trA[:, (b % 2) * 3 * D:(b % 2) * 3 * D + 3 * D]
                for i, s in enumerate(range(s0, s1)):
                    nc.tensor.transpose(pst[:, i * D:(i + 1) * D],
                                        kT[:, s * PC:(s + 1) * PC],
                                        ident[:D, :D])
                nc.vector.tensor_copy(ktm[:, s0 * D:s1 * D],
                                      pst[:, :(s1 - s0) * D])
                for s in range(s0, s1):
                    c = 15 + s
                    nc.tensor.matmul(csps[:, 2 * c:2 * c + 2],
                                     ktm[:, s * D:(s + 1) * D],
                                     v16[:, 2 * s:2 * s + 2],
                                     start=True, stop=True)
            def pre_transposes(b):
                # key-major h2 for prefix chunks, 3 per batch; G matmuls
                c0, c1 = 3 * b, min(3 * b + 3, NPRE)
                r = ((b + 1) % 2) * 3 * D
                pst = trA[:, r:r + 3 * D]
                for i, c in enumerate(range(c0, c1)):
                    j = 18 + c
                    nc.tensor.transpose(pst[:, i * D:(i + 1) * D],
                                        hk1[:, j * PC:(j + 1) * PC],
                                        ident[:D, :D])
                nc.vector.tensor_copy(h2t[:, c0 * D:c1 * D],
                                      pst[:, :(c1 - c0) * D])
                for c in range(c0, c1):
                    j = 18 + c
                    nc.tensor.matmul(gps[:, 2 * c:2 * c + 2],
                                     h2t[:, c * D:(c + 1) * D],
                                     v16[:, 2 * j:2 * j + 2],
                                     start=True, stop=True)
            def emit_prefix_csums():
                nc.vector.tensor_copy(gsb[:], gps)
                # prefix csums: csum_c = W3^T G_c + b3 * ssum_c
                for c in range(NPRE):
                    nc.tensor.matmul(csps[:, 2 * c:2 * c + 2],
                                     wsb[:, 5 * D:6 * D],
                                     gsb[:, 2 * c:2 * c + 2],
                                     start=True, stop=False)
                    nc.tensor.matmul(csps[:, 2 * c:2 * c + 2],
                                     b3row[:], ssb[:, 2 * c:2 * c + 2],
                                     start=False, stop=True)
                nc.vector.tensor_copy(csb[:, 0:30], csps[:, 0:30])
            # ---- explicit emission order (engine queues are in-order, so
            #      emission order is the per-engine schedule)
            srcs_k, dsts_k = [xk, hk0, hk1], [hk0, hk1, kT]
            srcs_q, dsts_q = [xq, hq0, hq1], [hq0, hq1, qT]
            def K(l, i, eng):
                c0, cw = kb[i]
                mlp_block(srcs_k[l], dsts_k[l], 3 + l, c0, cw, l < 2, eng)
            def Q(l, i, eng):
                c0, cw = qb[i]
                mlp_block(srcs_q[l], dsts_q[l], l, c0, cw, l < 2, eng)
            def out_av(lb):
                # window AV terms, accumulated as soon as the masks exist
                oslc = outp[:, 2 * lb:2 * lb + 2]
                for wo in range(3):
                    w = lb + wo
                    lb0 = max(0, w - 2)
                    nc.tensor.matmul(
                        oslc, mscs[w][:, (lb - lb0) * PC:(lb - lb0 + 1) * PC],
                        v16[:, 2 * w:2 * w + 2],
                        start=(wo == 0), stop=False)
            def out_prefix(lb):
                # prefix terms; closes the accumulation group
                oslc = outp[:, 2 * lb:2 * lb + 2]
                qb_ap = qT[:, lb * PC:(lb + 1) * PC]
                for c in range(lb + 15):
                    nc.tensor.matmul(oslc, qb_ap, csb[:, 2 * c:2 * c + 2],
                                     start=False, stop=(c == lb + 14))
                if lb % 4 == 3:
                    c0, c1 = 2 * lb - 6, 2 * lb + 2
                    nc.vector.tensor_copy(outb[:, c0:c1], outp[:, c0:c1])
                    nc.sync.dma_start(out_d[:, c0:c1], outb[:, c0:c1])
            # window path first; km/csum batches woven in (batches 5..9 =
            # window chunks 15..29, 0..4 = prefix chunks 0..14)
            K(0, 0, "d"); K(0, 1, "a"); K(0, 2, "d"); Q(0, 0, "a")
            K(0, 3, "d"); K(0, 4, "a"); Q(0, 1, "d"); K(0, 5, "a")
            K(1, 0, "d"); K(1, 1, "a"); Q(0, 2, "d"); K(1, 2, "a")
            nc.tensor.transpose(b3ps, bsb[:, 5:6], ident[:D, :D])
            nc.vector.tensor_copy(b3row[:], b3ps)
            km_batch(5, "d")
            Q(1, 0, "d"); K(1, 3, "a"); Q(0, 3, "d"); K(1, 4, "a")
            km_batch(6, "d")
            for c in range(30):
                j = (c - 15) if c >= 15 else (18 + c)
                nc.tensor.matmul(ssps[:, 2 * c:2 * c + 2], ones128[:],
                                 v16[:, 2 * j:2 * j + 2], start=True, stop=True)
            nc.vector.tensor_copy(ssb[:], ssps)
            Q(1, 1, "d"); K(1, 5, "a"); km_batch(7, "d")
            Q(2, 0, "a"); Q(1, 2, "a")
            K(2, 0, "a")
            score_mask(0); score_mask(1)
            km_batch(8, "d"); csum_batch(5, "d")
            Q(1, 3, "a"); Q(2, 1, "a")
            score_mask(2)
            km_batch(9, "d"); csum_batch(6, "d")
            out_av(0)
            K(2, 1, "a"); Q(2, 2, "a")
            score_mask(3); score_mask(4)
            csum_batch(7, "d")
            out_av(2)
            Q(2, 3, "a"); K(2, 2, "a")
            score_mask(5)
            csum_batch(8, "d")
            out_av(3)
            K(2, 3, "a")
            score_mask(6); score_mask(7)
            csum_batch(9, "d")
            out_av(5)
            K(2, 4, "a")
            score_mask(8)
            out_av(6)
            K(2, 5, "a")
            score_mask(9); score_mask(10)
            out_av(8)
            score_mask(11)
            out_av(9)
            score_mask(12); score_mask(13)
            out_av(11)
            score_mask(14)
            out_av(12)
            score_mask(15); score_mask(16); score_mask(17)
            out_av(15)
            # prefix chunks (Act wb; DVE is busy masking)
            K(0, 6, "a"); K(0, 7, "a"); K(0, 8, "a")
            K(0, 9, "a"); K(0, 10, "a")
            K(1, 6, "a"); km_batch(0, "a")
            K(1, 7, "a"); km_batch(1, "a")
            K(1, 8, "a"); km_batch(2, "a")
            K(1, 9, "a"); km_batch(3, "a")
            K(1, 10, "a"); km_batch(4, "a")
            for b in range(5):
                csum_batch(b, "a")
            for lb in range(QCH):
                out_prefix(lb)
    nc.compile()
    return nc

def _get_nc():
    if "nc" not in _NC_CACHE:
        _NC_CACHE["nc"] = _build_nc()
    return _NC_CACHE["nc"]

def kernel(x1, x2, x3, x4, wq, bq, wk, bk):
    xs = (x1, x2, x3, x4)
    if not _window_ok(x1, xs):
        return _numpy_fallback(x1, x2, x3, x4, wq, bq, wk, bk)
    in_maps = _shard_host(x1, x2, x3, x4, wq, bq, wk, bk)
    from concourse.bass_utils import run_bass_kernel_spmd
    nc = _get_nc()
    res = run_bass_kernel_spmd(nc, in_maps, list(range(8)))
    return _combine([r["out"] for r in res.results])


# revision 44
# speedup vs baseline: 1.0364x; 1.0364x over previous
"""Trainium2 Bass kernel for nn_CAMD_9990093930844 (sparse_attention).
Math: the reference computes, per modality m,
    out_m[i, :] = Q[i] @ S_m(t1[i]) ,  S_m(t) = sum_{j: t2_m[j] <= t} K_m[j] (x) V_m[j]
and returns (sum_m out_m)[:, :2].  Only V[:, :2] matters, so this is
    out[i, v] = sum_m sum_{j: t2_m[j] <= t1[i]} (Q[i] . K_m[j]) * V_m[j, v]
Both t1 and t2_m are sorted, so the rank deviation |p_m[i] - i| (p =
searchsorted) is bounded (~90 for this data).  Each 128-query block b
therefore only needs:
  - an unconditional prefix state over key chunks [0, b-1)
  - a masked local attention over key chunks {b-1, b, b+1}
Sharding: 8 cores = 4 modalities x 2 query halves, host sums the per-core
partial outputs.  Uniform SPMD program; pad chunks carry V=0 / t2=+inf so
they contribute nothing.
Device-program structure (all fp16 except timestamps / PSUM / output):
  - 3-layer MLPs for Q (2048 cols) and K (33 chunk-slots, window chunks
    first in the buffer).  PSUM->SBUF bias(+relu) writebacks are spread
    over Act / DVE / Pool.
  - Prefix-only chunks skip MLP layer 3: csum_c = W3^T (h2_c @ V_c)
    + b3 (sum_j V_cj)^T  (the "G-trick"), using a transpose of h2.
  - Window chunks: scores = kT_c^T qT (fp16, 1 cycle/row), timestamp mask
    fused on DVE/Pool into fp16 msc tiles.
  - All small matmuls are flipped so the moving operand has 2 columns
    (AV: stationary=msc chunk, moving=V; prefix apply: stationary=qT
    block, moving=csum/segment column) and accumulate per-block into one
    [128, 32] PSUM tile -> single copy -> single output DMA (q-major).
"""
import numpy as np
T = 4096
D = 66
M = 4
PC = 128                 # rows per chunk (partition dim)
QCH = 16                 # query blocks per core
KCH = 33                 # local key chunk slots per core
NWIN = 18                # window slots (local chunks 15..32)
NPRE = 15                # prefix-only chunks (local chunks 0..14)
QW = QCH * PC            # 2048 queries per core
KW = KCH * PC            # 4224 local keys per core
TBIG = 6.0e4             # timestamp sentinel for padded keys (> any real t, fp16-finite)
# buffer slot j -> local chunk index
PERM = list(range(15, 33)) + list(range(0, 15))

def _shard_host(x1, x2, x3, x4, wq, bq, wk, bk):
    """Build the 8 per-core input maps (host-side sharding/layout)."""
    xs = [np.asarray(x)[0, 0] for x in (x1, x2, x3, x4)]   # (4096, 66) each
    x1f = xs[0]
    wall = np.concatenate([np.asarray(wq), np.asarray(wk)], 0).astype(np.float32)
    ball = np.concatenate([np.asarray(bq), np.asarray(bk)], 0).astype(np.float32)
    wall_t = np.ascontiguousarray(
        wall.transpose(1, 0, 2).reshape(D, 6 * D)).astype(np.float16)
    ball_t = np.ascontiguousarray(ball.T).astype(np.float32)      # (66, 6)
    in_maps = []
    for core in range(8):
        m, h = core // 2, core % 2
        xm = xs[m]
        xq = np.ascontiguousarray(x1f[2048 * h: 2048 * h + 2048, :].T)
        # Local key chunks: local chunk lk = global chunk lk + 16*(h-1)
        g0 = 16 * (h - 1)
        kbuf = np.zeros((KW, D), np.float32)       # local-chunk order
        vbuf = np.zeros((KW, 2), np.float32)
        tbuf = np.full((KW,), TBIG, np.float32)
        lo_l = max(0, -g0)
        hi_l = min(KCH, 32 - g0)
        gl0 = (lo_l + g0) * PC
        gl1 = (hi_l + g0) * PC
        kbuf[lo_l * PC: hi_l * PC] = xm[gl0:gl1]
        vbuf[lo_l * PC: hi_l * PC] = xm[gl0:gl1, 0:2]
        tbuf[lo_l * PC: hi_l * PC] = xm[gl0:gl1, 65]
        # permute chunks into buffer-slot order (window chunks first)
        kc = kbuf.reshape(KCH, PC, D)[PERM].reshape(KW, D)
        vc = vbuf.reshape(KCH, PC, 2)[PERM]                     # (33,128,2)
        tc = tbuf.reshape(KCH, PC)[PERM]                        # (33,128)
        im = {
            "xq": np.ascontiguousarray(xq).astype(np.float16),
            "xk": np.ascontiguousarray(kc.T).astype(np.float16),
            "v16": np.ascontiguousarray(
                vc.transpose(1, 0, 2).reshape(PC, KCH * 2)).astype(np.float16),
            "t2s": np.ascontiguousarray(tc.T).astype(np.float16),   # (128,33)
            "t1b": np.ascontiguousarray(np.broadcast_to(
                x1f[2048 * h: 2048 * h + 2048, 65][None, :].astype(np.float16),
                (PC, QW))),
            "wall": wall_t,
            "ball": ball_t,
        }
        in_maps.append(im)
    return in_maps

def _window_ok(x1, xs):
    """Check the bounded-rank-deviation assumption the device program needs."""
    t1 = np.asarray(x1)[0, 0, :, 65]
    for xm in xs:
        t2 = np.asarray(xm)[0, 0, :, 65]
        p = np.searchsorted(t2, t1, side="right")
        b = np.arange(32)
        if not (p[b * PC] >= (b - 1) * PC).all():
            return False
        if not (p[b * PC + PC - 1] <= (b + 2) * PC).all():
            return False
    return True

def _nseg(lb):
    return min(3, (lb + 15) // 8)

def _core_emulate(im):
    """Numpy emulation of the device program for one core (validation)."""
    ws = [im["wall"].astype(np.float32)[:, i * D:(i + 1) * D] for i in range(6)]
    bs = [im["ball"].astype(np.float32)[:, i] for i in range(6)]
    def layer(x_T, w, b, relu):
        h = w.T @ x_T + b[:, None]
        return np.maximum(h, 0.0) if relu else h
    xq = im["xq"].astype(np.float32)
    xk = im["xk"].astype(np.float32)
    qT = layer(layer(layer(xq, ws[0], bs[0], 1), ws[1], bs[1], 1),
               ws[2], bs[2], 0)                                  # (66,2048)
    h2 = layer(layer(xk, ws[3], bs[3], 1), ws[4], bs[4], 1)      # (66,4224)
    kT_win = layer(h2[:, :NWIN * PC], ws[5], bs[5], 0)           # (66,2304)
    v = im["v16"].astype(np.float32).reshape(PC, KCH, 2).transpose(1, 0, 2)
    t2 = im["t2s"].astype(np.float32).T                           # (33,128)
    t1 = im["xq"][65].astype(np.float32)                          # (2048,)
    # chunk sums csum[c] for local chunks c = 0..29
    csum = np.zeros((30, D, 2), np.float32)
    for c in range(30):
        if c >= 15:
            w = c - 15                       # window slot
            csum[c] = kT_win[:, w * PC:(w + 1) * PC] @ v[w]
        else:
            j = 18 + c                       # prefix slot
            G = h2[:, j * PC:(j + 1) * PC] @ v[j]                 # (66,2)
            csum[c] = ws[5].T @ G + bs[5][:, None] * v[j].sum(0)[None, :]
    seg = np.stack([csum[0:8].sum(0), csum[8:16].sum(0), csum[16:24].sum(0)])
    out = np.zeros((PC, QCH * 2), np.float32)
    for lb in range(QCH):
        qb = qT[:, lb * PC:(lb + 1) * PC]                        # (66,128)
        acc = np.zeros((PC, 2), np.float32)
        ns = _nseg(lb)
        for s in range(ns):
            acc += qb.T @ seg[s]
        for c in range(8 * ns, lb + 15):
            acc += qb.T @ csum[c]
        for wo in range(3):
            w = lb + wo                      # window slot
            sc = kT_win[:, w * PC:(w + 1) * PC].T @ qb           # (128k,128q)
            msk = (t1[None, lb * PC:(lb + 1) * PC] >=
                   t2[w][:, None]).astype(np.float32)
            acc += (sc * msk).T @ v[w]
        out[:, 2 * lb:2 * lb + 2] = acc
    return out                                                    # (128,32)

def _combine(per_core_outs):
    full = np.zeros((T, 2), np.float32)
    for core, o in enumerate(per_core_outs):
        h = core % 2
        o = np.asarray(o).reshape(PC, QCH, 2)
        full[2048 * h: 2048 * h + 2048] += \
            o.transpose(1, 0, 2).reshape(QW, 2)
    return full[None, :, :]

def _numpy_fallback(x1, x2, x3, x4, wq, bq, wk, bk):
    """Exact dense fallback (used only if the window assumption fails)."""
    xs = [np.asarray(x)[0, 0].astype(np.float64) for x in (x1, x2, x3, x4)]
    def mlp(x, W, b):
        h = x
        for l in range(2):
            h = np.maximum(h @ W[l] + b[l], 0.0)
        return h @ W[2] + b[2]
    Q = mlp(xs[0], np.asarray(wq, np.float64), np.asarray(bq, np.float64))
    t1 = xs[0][:, 65]
    out = np.zeros((T, 2))
    for m in range(M):
        Km = mlp(xs[m], np.asarray(wk, np.float64), np.asarray(bk, np.float64))
        t2 = xs[m][:, 65]
        mask = t2[None, :] <= t1[:, None]
        A = (Q @ Km.T) * mask
        out += A @ xs[m][:, 0:2]
    return out[None].astype(np.float32)

# ---------------------------------------------------------------------------
# Bass device program
# ---------------------------------------------------------------------------
_NC_CACHE = {}

def _build_nc():
    import concourse.bacc as bacc
    import concourse.mybir as mybir
    import concourse.tile as tile
    from concourse import masks
    f32 = mybir.dt.float32
    f16 = mybir.dt.float16
    AF = mybir.ActivationFunctionType
    ALU = mybir.AluOpType
    nc = bacc.Bacc("TRN2", target_bir_lowering=False, debug=False,
                   enable_asserts=False, num_devices=8)
    xq_d = nc.dram_tensor("xq", [D, QW], f16, kind="ExternalInput")
    xk_d = nc.dram_tensor("xk", [D, KW], f16, kind="ExternalInput")
    v16_d = nc.dram_tensor("v16", [PC, KCH * 2], f16, kind="ExternalInput")
    t2s_d = nc.dram_tensor("t2s", [PC, KCH], f16, kind="ExternalInput")
    t1b_d = nc.dram_tensor("t1b", [PC, QW], f16, kind="ExternalInput")
    wall_d = nc.dram_tensor("wall", [D, 6 * D], f16, kind="ExternalInput")
    ball_d = nc.dram_tensor("ball", [D, 6], f32, kind="ExternalInput")
    out_d = nc.dram_tensor("out", [PC, QCH * 2], f32, kind="ExternalOutput")
    # elementwise-engine weights for MLP writebacks (per 15 pieces:
    # Act 10, DVE 3, Pool 2 -- matches engine col/ns rates)
    def wb_eng_for(kind, l, i):
        if l == 2:
            return "a"
        if kind == "k":
            if i >= 6:                      # prefix chunks (late, Act)
                return "a"
            return "d" if i % 2 == 0 else "a"
        return "d" if i in (1, 3) else "a"  # Q l0/l1
    # mask engine per window slot: "d" = DVE STT straight from PSUM,
    # "p" = Act copies scores to SBUF, Pool does the STT (GPSIMD cannot
    # read PSUM on real hardware)
    mask_eng = ["d"] * 18
    with tile.TileContext(nc) as tc:
        with (
            tc.tile_pool(name="const", bufs=1) as cpool,
            tc.tile_pool(name="big", bufs=1) as bpool,
            tc.tile_pool(name="ps_mlp", bufs=4, space="PSUM") as ps_mlp,
            tc.tile_pool(name="ps_sc", bufs=2, space="PSUM") as ps_sc,
            tc.tile_pool(name="ps_tr", bufs=1, space="PSUM") as ps_tr,
            tc.tile_pool(name="ps_sm", bufs=1, space="PSUM") as ps_sm,
        ):
            # ---- tiles
            wsb = cpool.tile([D, 6 * D], f16)
            bsb = cpool.tile([D, 6], f32)
            b3row = cpool.tile([1, D], f16)
            xq = bpool.tile([D, QW], f16)
            xk = bpool.tile([D, KW], f16)
            v16 = bpool.tile([PC, KCH * 2], f16)
            t2s = bpool.tile([PC, KCH], f16)
            t1b = bpool.tile([PC, QW], f16)
            # ---- input DMAs, ordered by first use (t1b is built on-device
            #      by a Pool partition-broadcast of xq row 65)
            nc.sync.dma_start(wsb[:], wall_d[:])
            nc.sync.dma_start(xk[:, 0:384], xk_d[:, 0:384])
            nc.sync.dma_start(xq[:, 0:512], xq_d[:, 0:512])
            nc.sync.dma_start(xk[:, 384:2304], xk_d[:, 384:2304])
            nc.sync.dma_start(xq[:, 512:2048], xq_d[:, 512:2048])
            nc.sync.dma_start(xk[:, 2304:4224], xk_d[:, 2304:4224])
            nc.sync.dma_start(v16[:], v16_d[:])
            nc.sync.dma_start(t2s[:], t2s_d[:])
            nc.sync.dma_start(t1b[:, 0:1024], t1b_d[:, 0:1024])
            nc.sync.dma_start(t1b[:, 1024:2048], t1b_d[:, 1024:2048])
            nc.scalar.dma_start(bsb[:], ball_d[:])
            ident = cpool.tile([128, 128], f32)
            masks.make_identity(nc, ident[:])
            ones128 = cpool.tile([128, 1], f16)
            nc.gpsimd.memset(ones128[:], 1.0)
            # single PSUM bank carved into the small accumulation regions
            smA = ps_sm.tile([128, 512], f32, name="smA")
            outp = smA[:, 0:QCH * 2]
            ssps = smA[:1, 32:92]
            csps = smA[:D, 92:152]
            # ---- PE p-state warmup (outputs unused)
            trA = ps_tr.tile([PC, 2 * 3 * D], f32, name="trA")
            for i in range(12):
                nc.tensor.transpose(trA[:, 0:PC], ident[:], ident[:])
            b3ps = smA[:1, 152:152 + D]
            # ---- MLPs --------------------------------------------------
            hq0 = bpool.tile([D, QW], f16)
            hq1 = bpool.tile([D, QW], f16)
            qT = bpool.tile([D, QW], f16)
            hk0 = bpool.tile([D, KW], f16)
            hk1 = bpool.tile([D, KW], f16)
            kT = bpool.tile([D, NWIN * PC], f16)
            def wb(eng, dst, ps_ap, b_ap, relu):
                if eng == "a":
                    nc.scalar.activation(dst, ps_ap,
                                         AF.Relu if relu else AF.Identity,
                                         bias=b_ap)
                elif eng == "d":
                    if relu:
                        nc.vector.tensor_scalar(dst, ps_ap, b_ap, 0.0,
                                                ALU.add, ALU.max)
                    else:
                        nc.vector.tensor_scalar_add(dst, ps_ap, b_ap)
                else:
                    if relu:
                        nc.gpsimd.tensor_scalar(dst, ps_ap, b_ap, 0.0,
                                                ALU.add, ALU.max)
                    else:
                        nc.gpsimd.tensor_scalar_add(dst, ps_ap, b_ap)
            def mlp_block(src, dst, wofs, c0, cw, relu, wb_eng="a"):
                w_ap = wsb[:, wofs * D:(wofs + 1) * D]
                b_ap = bsb[:, wofs:wofs + 1]
                ps = ps_mlp.tile([D, 512], f32, tag="mlp",
                                 name=f"mlp{wofs}_{c0}")
                nc.tensor.matmul(ps[:, :cw], w_ap, src[:, c0:c0 + cw],
                                 start=True, stop=True)
                eng = wb_eng
                wb(eng, dst[:, c0:c0 + cw], ps[:, :cw], b_ap, relu)
            # K blocks: 11 x 384 (blocks 0..5 = window slots, 6..10 = prefix)
            # Q blocks: 4 x 512
            kb = [(i * 384, 384) for i in range(11)]
            qb = [(i * 512, 512) for i in range(4)]
            # ---- downstream machinery ---------------------------------
            km = bpool.tile([PC, 30 * D], f16)   # key-major K, chunks 0..29
            csb = cpool.tile([D, 60], f16)
            ssb = cpool.tile([1, 60], f16)
            mscs = []
            for w in range(NWIN):
                mscs.append(bpool.tile([PC, 384], f16, name=f"msc{w}"))
            outb = bpool.tile([PC, QCH * 2], f32)
            def score_mask(w):
                lb0 = max(0, w - 2)
                lb1 = min(QCH - 1, w)
                ncol = (lb1 - lb0 + 1) * PC
                ps = ps_sc.tile([PC, 384], f32, tag="sc", name=f"sc{w}")
                nc.tensor.matmul(ps[:, :ncol], kT[:, w * PC:(w + 1) * PC],
                                 qT[:, lb0 * PC:(lb1 + 1) * PC],
                                 start=True, stop=True)
                if mask_eng[w] == "d":
                    nc.vector.scalar_tensor_tensor(
                        mscs[w][:, :ncol],
                        t1b[:, lb0 * PC:(lb1 + 1) * PC],
                        t2s[:, w:w + 1],
                        ps[:, :ncol],
                        ALU.is_ge, ALU.mult)
                else:
                    scb = bpool.tile([PC, 384], f16, name=f"scb{w}")
                    nc.scalar.copy(scb[:, :ncol], ps[:, :ncol])
                    nc.gpsimd.scalar_tensor_tensor(
                        mscs[w][:, :ncol],
                        t1b[:, lb0 * PC:(lb1 + 1) * PC],
                        t2s[:, w:w + 1],
                        scb[:, :ncol],
                        ALU.is_ge, ALU.mult)
            def win_transposes(b):
                # key-major K for csum chunks 15..29 (slots 0..14), 3 per batch
                s0, s1 = 3 * b, min(3 * b + 3, NPRE)
                pst = trA[:, (b % 2) * 3 * D:(b % 2) * 3 * D + 3 * D]
                for i, s in enumerate(range(s0, s1)):
                    nc.tensor.transpose(pst[:, i * D:(i + 1) * D],
                                        kT[:, s * PC:(s + 1) * PC],
                                        ident[:D, :D])
                nc.vector.tensor_copy(ktm[:, s0 * D:s1 * D],
                                      pst[:, :(s1 - s0) * D])
                for s in range(s0, s1):
                    c = 15 + s
                    nc.tensor.matmul(csps[:, 2 * c:2 * c + 2],
                                     ktm[:, s * D:(s + 1) * D],
                                     v16[:, 2 * s:2 * s + 2],
                                     start=True, stop=True)
            def pre_transposes(b):
                # key-major h2 for prefix chunks, 3 per batch; G matmuls
                c0, c1 = 3 * b, min(3 * b + 3, NPRE)
                r = ((b + 1) % 2) * 3 * D
                pst = trA[:, r:r + 3 * D]
                for i, c in enumerate(range(c0, c1)):
                    j = 18 + c
                    nc.tensor.transpose(pst[:, i * D:(i + 1) * D],
                                        hk1[:, j * PC:(j + 1) * PC],
                                        ident[:D, :D])
                nc.vector.tensor_copy(h2t[:, c0 * D:c1 * D],
                                      pst[:, :(c1 - c0) * D])
                for c in range(c0, c1):
                    j = 18 + c
                    nc.tensor.matmul(gps[:, 2 * c:2 * c + 2],
                                     h2t[:, c * D:(c + 1) * D],
                                     v16[:, 2 * j:2 * j + 2],
                                     start=True, stop=True)
            def emit_prefix_csums():
                nc.vector.tensor_copy(gsb[:], gps)
                # prefix csums: csum_c = W3^T G_c + b3 * ssum_c
                for c in range(NPRE):
                    nc.tensor.matmul(csps[:, 2 * c:2 * c + 2],
                                     wsb[:, 5 * D:6 * D],
                                     gsb[:, 2 * c:2 * c + 2],
                                     start=True, stop=False)
                    nc.tensor.matmul(csps[:, 2 * c:2 * c + 2],
                                     b3row[:], ssb[:, 2 * c:2 * c + 2],
                                     start=False, stop=True)
                nc.vector.tensor_copy(csb[:, 0:30], csps[:, 0:30])
            # ---- explicit emission order (engine queues are in-order, so
            #      emission order is the per-engine schedule)
            srcs_k, dsts_k = [xk, hk0, hk1], [hk0, hk1, kT]
            srcs_q, dsts_q = [xq, hq0, hq1], [hq0, hq1, qT]
            def K(l, i, eng):
                c0, cw = kb[i]
                mlp_block(srcs_k[l], dsts_k[l], 3 + l, c0, cw, l < 2, eng)
            def Q(l, i, eng):
                c0, cw = qb[i]
                mlp_block(srcs_q[l], dsts_q[l], l, c0, cw, l < 2, eng)
            def out_av(lb):
                # window AV terms, accumulated as soon as the masks exist
                oslc = outp[:, 2 * lb:2 * lb + 2]
                for wo in range(3):
                    w = lb + wo
                    lb0 = max(0, w - 2)
                    nc.tensor.matmul(
                        oslc, mscs[w][:, (lb - lb0) * PC:(lb - lb0 + 1) * PC],
                        v16[:, 2 * w:2 * w + 2],
                        start=(wo == 0), stop=False)
            def out_prefix(lb):
                # prefix terms; closes the accumulation group
                oslc = outp[:, 2 * lb:2 * lb + 2]
                qb_ap = qT[:, lb * PC:(lb + 1) * PC]
                for c in range(lb + 15):
                    nc.tensor.matmul(oslc, qb_ap, csb[:, 2 * c:2 * c + 2],
                                     start=False, stop=(c == lb + 14))
                if lb % 4 == 3:
                    c0, c1 = 2 * lb - 6, 2 * lb + 2
                    nc.vector.tensor_copy(outb[:, c0:c1], outp[:, c0:c1])
                    nc.sync.dma_start(out_d[:, c0:c1], outb[:, c0:c1])
            # window path first; km/csum batches woven in (batches 5..9 =
            # window chunks 15..29, 0..4 = prefix chunks 0..14)
            K(0, 0, "d"); K(0, 1, "a"); K(0, 2, "d"); Q(0, 0, "a")
            K(0, 3, "d"); K(0, 4, "a"); Q(0, 1, "d"); K(0, 5, "a")
            K(1, 0, "d"); K(1, 1, "a"); Q(0, 2, "d"); K(1, 2, "a")
            nc.tensor.transpose(b3ps, bsb[:, 5:6], ident[:D, :D])
            nc.vector.tensor_copy(b3row[:], b3ps)
            km_batch(5, "d")
            Q(1, 0, "d"); K(1, 3, "a"); Q(0, 3, "d"); K(1, 4, "a")
            km_batch(6, "d")
            for c in range(30):
                j = (c - 15) if c >= 15 else (18 + c)
                nc.tensor.matmul(ssps[:, 2 * c:2 * c + 2], ones128[:],
                                 v16[:, 2 * j:2 * j + 2], start=True, stop=True)
            nc.vector.tensor_copy(ssb[:], ssps)
            Q(1, 1, "d"); K(1, 5, "a"); km_batch(7, "d")
            Q(2, 0, "a"); Q(1, 2, "a")
            K(2, 0, "a")
            score_mask(0); score_mask(1)
            km_batch(8, "d"); csum_batch(5, "d")
            Q(1, 3, "a"); Q(2, 1, "a")
            score_mask(2)
            km_batch(9, "d"); csum_batch(6, "d")
            out_av(0)
            K(2, 1, "a"); Q(2, 2, "a")
            score_mask(3); score_mask(4)
            csum_batch(7, "d")
            out_av(2)
            Q(2, 3, "a"); K(2, 2, "a")
            score_mask(5)
            csum_batch(8, "d")
            out_av(3)
            K(2, 3, "a")
            score_mask(6); score_mask(7)
            csum_batch(9, "d")
            out_av(5)
            K(2, 4, "a")
            score_mask(8)
            out_av(6)
            K(2, 5, "a")
            score_mask(9); score_mask(10)
            out_av(8)
            score_mask(11)
            out_av(9)
            score_mask(12); score_mask(13)
            out_av(11)
            score_mask(14)
            out_av(12)
            score_mask(15); score_mask(16); score_mask(17)
            out_av(15)
            # prefix chunks (Act wb; DVE is busy masking)
            K(0, 6, "a"); K(0, 7, "a"); K(0, 8, "a")
            K(0, 9, "a"); K(0, 10, "a")
            K(1, 6, "a"); km_batch(0, "d")
            K(1, 7, "a"); km_batch(1, "d")
            K(1, 8, "a"); km_batch(2, "d")
            K(1, 9, "a"); km_batch(3, "d")
            K(1, 10, "a"); km_batch(4, "d")
            for b in range(5):
                csum_batch(b, "d")
            for lb in range(QCH):
                out_prefix(lb)
    nc.compile()
    return nc

def _get_nc():
    if "nc" not in _NC_CACHE:
        _NC_CACHE["nc"] = _build_nc()
    return _NC_CACHE["nc"]

def kernel(x1, x2, x3, x4, wq, bq, wk, bk):
    xs = (x1, x2, x3, x4)
    if not _window_ok(x1, xs):
        return _numpy_fallback(x1, x2, x3, x4, wq, bq, wk, bk)
    in_maps = _shard_host(x1, x2, x3, x4, wq, bq, wk, bk)
    from concourse.bass_utils import run_bass_kernel_spmd
    nc = _get_nc()
    res = run_bass_kernel_spmd(nc, in_maps, list(range(8)))
    return _combine([r["out"] for r in res.results])


# revision 47
# speedup vs baseline: 1.1285x; 1.0888x over previous
"""Trainium2 Bass kernel for nn_CAMD_9990093930844 (sparse_attention).
Math: the reference computes, per modality m,
    out_m[i, :] = Q[i] @ S_m(t1[i]) ,  S_m(t) = sum_{j: t2_m[j] <= t} K_m[j] (x) V_m[j]
and returns (sum_m out_m)[:, :2].  Only V[:, :2] matters, so this is
    out[i, v] = sum_m sum_{j: t2_m[j] <= t1[i]} (Q[i] . K_m[j]) * V_m[j, v]
Both t1 and t2_m are sorted, so the rank deviation |p_m[i] - i| (p =
searchsorted) is bounded (~90 for this data).  Each 128-query block b
therefore only needs:
  - an unconditional prefix state over key chunks [0, b-1)
  - a masked local attention over key chunks {b-1, b, b+1}
Sharding: 8 cores = 4 modalities x 2 query halves, host sums the per-core
partial outputs.  Uniform SPMD program; pad chunks carry V=0 / t2=+inf so
they contribute nothing.
Device-program structure (all fp16 except timestamps / PSUM / output):
  - 3-layer MLPs for Q (2048 cols) and K (33 chunk-slots, window chunks
    first in the buffer).  PSUM->SBUF bias(+relu) writebacks are spread
    over Act / DVE / Pool.
  - Prefix-only chunks skip MLP layer 3: csum_c = W3^T (h2_c @ V_c)
    + b3 (sum_j V_cj)^T  (the "G-trick"), using a transpose of h2.
  - Window chunks: scores = kT_c^T qT (fp16, 1 cycle/row), timestamp mask
    fused on DVE/Pool into fp16 msc tiles.
  - All small matmuls are flipped so the moving operand has 2 columns
    (AV: stationary=msc chunk, moving=V; prefix apply: stationary=qT
    block, moving=csum/segment column) and accumulate per-block into one
    [128, 32] PSUM tile -> single copy -> single output DMA (q-major).
"""
import numpy as np
T = 4096
D = 66
M = 4
PC = 128                 # rows per chunk (partition dim)
QCH = 16                 # query blocks per core
KCH = 33                 # local key chunk slots per core
NWIN = 18                # window slots (local chunks 15..32)
NPRE = 15                # prefix-only chunks (local chunks 0..14)
QW = QCH * PC            # 2048 queries per core
KW = KCH * PC            # 4224 local keys per core
TBIG = 6.0e4             # timestamp sentinel for padded keys (> any real t, fp16-finite)
# buffer slot j -> local chunk index
PERM = list(range(15, 33)) + list(range(0, 15))

def _shard_host(x1, x2, x3, x4, wq, bq, wk, bk):
    """Build the 8 per-core input maps (host-side sharding/layout)."""
    xs = [np.asarray(x)[0, 0] for x in (x1, x2, x3, x4)]   # (4096, 66) each
    x1f = xs[0]
    wall = np.concatenate([np.asarray(wq), np.asarray(wk)], 0).astype(np.float32)
    ball = np.concatenate([np.asarray(bq), np.asarray(bk)], 0).astype(np.float32)
    wall_t = np.ascontiguousarray(
        wall.transpose(1, 0, 2).reshape(D, 6 * D)).astype(np.float16)
    ball_t = np.ascontiguousarray(ball.T).astype(np.float32)      # (66, 6)
    in_maps = []
    for core in range(8):
        m, h = core // 2, core % 2
        xm = xs[m]
        xq = np.ascontiguousarray(x1f[2048 * h: 2048 * h + 2048, :].T)
        # Local key chunks: local chunk lk = global chunk lk + 16*(h-1)
        g0 = 16 * (h - 1)
        kbuf = np.zeros((KW, D), np.float32)       # local-chunk order
        vbuf = np.zeros((KW, 2), np.float32)
        tbuf = np.full((KW,), TBIG, np.float32)
        lo_l = max(0, -g0)
        hi_l = min(KCH, 32 - g0)
        gl0 = (lo_l + g0) * PC
        gl1 = (hi_l + g0) * PC
        kbuf[lo_l * PC: hi_l * PC] = xm[gl0:gl1]
        vbuf[lo_l * PC: hi_l * PC] = xm[gl0:gl1, 0:2]
        tbuf[lo_l * PC: hi_l * PC] = xm[gl0:gl1, 65]
        # permute chunks into buffer-slot order (window chunks first)
        kc = kbuf.reshape(KCH, PC, D)[PERM].reshape(KW, D)
        vc = vbuf.reshape(KCH, PC, 2)[PERM]                     # (33,128,2)
        tc = tbuf.reshape(KCH, PC)[PERM]                        # (33,128)
        im = {
            "xq": np.ascontiguousarray(xq).astype(np.float16),
            "xk": np.ascontiguousarray(kc.T).astype(np.float16),
            "v16": np.ascontiguousarray(
                vc.transpose(1, 0, 2).reshape(PC, KCH * 2)).astype(np.float16),
            "t2s": np.ascontiguousarray(tc.T).astype(np.float16),   # (128,33)
            "t1b": np.ascontiguousarray(np.broadcast_to(
                x1f[2048 * h: 2048 * h + 2048, 65][None, :].astype(np.float16),
                (PC, QW))),
            "wall": wall_t,
            "ball": ball_t,
        }
        in_maps.append(im)
    return in_maps

def _window_ok(x1, xs):
    """Check the bounded-rank-deviation assumption the device program needs."""
    t1 = np.asarray(x1)[0, 0, :, 65]
    for xm in xs:
        t2 = np.asarray(xm)[0, 0, :, 65]
        p = np.searchsorted(t2, t1, side="right")
        b = np.arange(32)
        if not (p[b * PC] >= (b - 1) * PC).all():
            return False
        if not (p[b * PC + PC - 1] <= (b + 2) * PC).all():
            return False
    return True

def _nseg(lb):
    return min(3, (lb + 15) // 8)

def _core_emulate(im):
    """Numpy emulation of the device program for one core (validation)."""
    ws = [im["wall"].astype(np.float32)[:, i * D:(i + 1) * D] for i in range(6)]
    bs = [im["ball"].astype(np.float32)[:, i] for i in range(6)]
    def layer(x_T, w, b, relu):
        h = w.T @ x_T + b[:, None]
        return np.maximum(h, 0.0) if relu else h
    xq = im["xq"].astype(np.float32)
    xk = im["xk"].astype(np.float32)
    qT = layer(layer(layer(xq, ws[0], bs[0], 1), ws[1], bs[1], 1),
               ws[2], bs[2], 0)                                  # (66,2048)
    h2 = layer(layer(xk, ws[3], bs[3], 1), ws[4], bs[4], 1)      # (66,4224)
    kT_win = layer(h2[:, :NWIN * PC], ws[5], bs[5], 0)           # (66,2304)
    v = im["v16"].astype(np.float32).reshape(PC, KCH, 2).transpose(1, 0, 2)
    t2 = im["t2s"].astype(np.float32).T                           # (33,128)
    t1 = im["xq"][65].astype(np.float32)                          # (2048,)
    # chunk sums csum[c] for local chunks c = 0..29
    csum = np.zeros((30, D, 2), np.float32)
    for c in range(30):
        if c >= 15:
            w = c - 15                       # window slot
            csum[c] = kT_win[:, w * PC:(w + 1) * PC] @ v[w]
        else:
            j = 18 + c                       # prefix slot
            G = h2[:, j * PC:(j + 1) * PC] @ v[j]                 # (66,2)
            csum[c] = ws[5].T @ G + bs[5][:, None] * v[j].sum(0)[None, :]
    seg = np.stack([csum[0:8].sum(0), csum[8:16].sum(0), csum[16:24].sum(0)])
    out = np.zeros((PC, QCH * 2), np.float32)
    for lb in range(QCH):
        qb = qT[:, lb * PC:(lb + 1) * PC]                        # (66,128)
        acc = np.zeros((PC, 2), np.float32)
        ns = _nseg(lb)
        for s in range(ns):
            acc += qb.T @ seg[s]
        for c in range(8 * ns, lb + 15):
            acc += qb.T @ csum[c]
        for wo in range(3):
            w = lb + wo                      # window slot
            sc = kT_win[:, w * PC:(w + 1) * PC].T @ qb           # (128k,128q)
            msk = (t1[None, lb * PC:(lb + 1) * PC] >=
                   t2[w][:, None]).astype(np.float32)
            acc += (sc * msk).T @ v[w]
        out[:, 2 * lb:2 * lb + 2] = acc
    return out                                                    # (128,32)

def _combine(per_core_outs):
    full = np.zeros((T, 2), np.float32)
    for core, o in enumerate(per_core_outs):
        h = core % 2
        o = np.asarray(o).reshape(PC, QCH, 2)
        full[2048 * h: 2048 * h + 2048] += \
            o.transpose(1, 0, 2).reshape(QW, 2)
    return full[None, :, :]

def _numpy_fallback(x1, x2, x3, x4, wq, bq, wk, bk):
    """Exact dense fallback (used only if the window assumption fails)."""
    xs = [np.asarray(x)[0, 0].astype(np.float64) for x in (x1, x2, x3, x4)]
    def mlp(x, W, b):
        h = x
        for l in range(2):
            h = np.maximum(h @ W[l] + b[l], 0.0)
        return h @ W[2] + b[2]
    Q = mlp(xs[0], np.asarray(wq, np.float64), np.asarray(bq, np.float64))
    t1 = xs[0][:, 65]
    out = np.zeros((T, 2))
    for m in range(M):
        Km = mlp(xs[m], np.asarray(wk, np.float64), np.asarray(bk, np.float64))
        t2 = xs[m][:, 65]
        mask = t2[None, :] <= t1[:, None]
        A = (Q @ Km.T) * mask
        out += A @ xs[m][:, 0:2]
    return out[None].astype(np.float32)

# ---------------------------------------------------------------------------
# Bass device program
# ---------------------------------------------------------------------------
_NC_CACHE = {}

def _build_nc():
    import concourse.bacc as bacc
    import concourse.mybir as mybir
    import concourse.tile as tile
    from concourse import masks
    f32 = mybir.dt.float32
    f16 = mybir.dt.float16
    AF = mybir.ActivationFunctionType
    ALU = mybir.AluOpType
    nc = bacc.Bacc("TRN2", target_bir_lowering=False, debug=False,
                   enable_asserts=False, num_devices=8)
    xq_d = nc.dram_tensor("xq", [D, QW], f16, kind="ExternalInput")
    xk_d = nc.dram_tensor("xk", [D, KW], f16, kind="ExternalInput")
    v16_d = nc.dram_tensor("v16", [PC, KCH * 2], f16, kind="ExternalInput")
    t2s_d = nc.dram_tensor("t2s", [PC, KCH], f16, kind="ExternalInput")
    t1b_d = nc.dram_tensor("t1b", [PC, QW], f16, kind="ExternalInput")
    wall_d = nc.dram_tensor("wall", [D, 6 * D], f16, kind="ExternalInput")
    ball_d = nc.dram_tensor("ball", [D, 6], f32, kind="ExternalInput")
    out_d = nc.dram_tensor("out", [PC, QCH * 2], f32, kind="ExternalOutput")
    # elementwise-engine weights for MLP writebacks (per 15 pieces:
    # Act 10, DVE 3, Pool 2 -- matches engine col/ns rates)
    def wb_eng_for(kind, l, i):
        if l == 2:
            return "a"
        if kind == "k":
            if i >= 6:                      # prefix chunks (late, Act)
                return "a"
            return "d" if i % 2 == 0 else "a"
        return "d" if i in (1, 3) else "a"  # Q l0/l1
    # mask engine per window slot: "d" = DVE STT straight from PSUM,
    # "p" = Act copies scores to SBUF, Pool does the STT (GPSIMD cannot
    # read PSUM on real hardware)
    mask_eng = ["d"] * 18
    with tile.TileContext(nc) as tc:
        with (
            tc.tile_pool(name="const", bufs=1) as cpool,
            tc.tile_pool(name="big", bufs=1) as bpool,
            tc.tile_pool(name="ps_mlp", bufs=4, space="PSUM") as ps_mlp,
            tc.tile_pool(name="ps_sc", bufs=2, space="PSUM") as ps_sc,
            tc.tile_pool(name="ps_tr", bufs=1, space="PSUM") as ps_tr,
            tc.tile_pool(name="ps_sm", bufs=1, space="PSUM") as ps_sm,
        ):
            # ---- tiles
            wsb = cpool.tile([D, 6 * D], f16)
            bsb = cpool.tile([D, 6], f32)
            b3row = cpool.tile([1, D], f16)
            xq = bpool.tile([D, QW], f16)
            xk = bpool.tile([D, KW], f16)
            v16 = bpool.tile([PC, KCH * 2], f16)
            t2s = bpool.tile([PC, KCH], f16)
            t1b = bpool.tile([PC, QW], f16)
            # ---- input DMAs, ordered by first use (t1b is built on-device
            #      by a Pool partition-broadcast of xq row 65)
            nc.sync.dma_start(wsb[:], wall_d[:])
            nc.sync.dma_start(xk[:, 0:384], xk_d[:, 0:384])
            nc.sync.dma_start(xq[:, 0:512], xq_d[:, 0:512])
            nc.sync.dma_start(xk[:, 384:2304], xk_d[:, 384:2304])
            nc.sync.dma_start(xq[:, 512:2048], xq_d[:, 512:2048])
            nc.sync.dma_start(xk[:, 2304:4224], xk_d[:, 2304:4224])
            nc.sync.dma_start(v16[:], v16_d[:])
            nc.sync.dma_start(t2s[:], t2s_d[:])
            nc.sync.dma_start(t1b[:, 0:1024], t1b_d[:, 0:1024])
            nc.sync.dma_start(t1b[:, 1024:2048], t1b_d[:, 1024:2048])
            nc.scalar.dma_start(bsb[:], ball_d[:])
            ident = cpool.tile([128, 128], f32)
            masks.make_identity(nc, ident[:])
            ones128 = cpool.tile([128, 1], f16)
            nc.gpsimd.memset(ones128[:], 1.0)
            # single PSUM bank carved into the small accumulation regions
            smA = ps_sm.tile([128, 512], f32, name="smA")
            outp = smA[:, 0:QCH * 2]
            ssps = smA[:1, 32:92]
            csps = smA[:D, 92:152]
            # ---- PE p-state warmup (outputs unused)
            trA = ps_tr.tile([PC, 2 * 3 * D], f32, name="trA")
            for i in range(12):
                nc.tensor.transpose(trA[:, 0:PC], ident[:], ident[:])
            b3ps = smA[:1, 152:152 + D]
            # ---- MLPs --------------------------------------------------
            hq0 = bpool.tile([D, QW], f16)
            hq1 = bpool.tile([D, QW], f16)
            qT = bpool.tile([D, QW], f16)
            hk0 = bpool.tile([D, KW], f16)
            hk1 = bpool.tile([D, KW], f16)
            kT = bpool.tile([D, NWIN * PC], f16)
            def wb(eng, dst, ps_ap, b_ap, relu):
                if eng == "a":
                    nc.scalar.activation(dst, ps_ap,
                                         AF.Relu if relu else AF.Identity,
                                         bias=b_ap)
                elif eng == "d":
                    if relu:
                        nc.vector.tensor_scalar(dst, ps_ap, b_ap, 0.0,
                                                ALU.add, ALU.max)
                    else:
                        nc.vector.tensor_scalar_add(dst, ps_ap, b_ap)
                else:
                    if relu:
                        nc.gpsimd.tensor_scalar(dst, ps_ap, b_ap, 0.0,
                                                ALU.add, ALU.max)
                    else:
                        nc.gpsimd.tensor_scalar_add(dst, ps_ap, b_ap)
            def mlp_block(src, dst, wofs, c0, cw, relu, wb_eng="a"):
                w_ap = wsb[:, wofs * D:(wofs + 1) * D]
                b_ap = bsb[:, wofs:wofs + 1]
                ps = ps_mlp.tile([D, 512], f32, tag="mlp",
                                 name=f"mlp{wofs}_{c0}")
                nc.tensor.matmul(ps[:, :cw], w_ap, src[:, c0:c0 + cw],
                                 start=True, stop=True)
                eng = wb_eng
                wb(eng, dst[:, c0:c0 + cw], ps[:, :cw], b_ap, relu)
            # K blocks: 11 x 384 (blocks 0..5 = window slots, 6..10 = prefix)
            # Q blocks: 4 x 512
            kb = [(i * 384, 384) for i in range(11)]
            qb = [(i * 512, 512) for i in range(4)]
            # ---- downstream machinery ---------------------------------
            km = bpool.tile([PC, 30 * D], f16)   # key-major K, chunks 0..29
            csb = cpool.tile([D, 60], f16)
            ssb = cpool.tile([1, 60], f16)
            mscs = []
            for w in range(NWIN):
                mscs.append(bpool.tile([PC, 384], f16, name=f"msc{w}"))
            outb = bpool.tile([PC, QCH * 2], f32)
            def score_mask(w):
                lb0 = max(0, w - 2)
                lb1 = min(QCH - 1, w)
                ncol = (lb1 - lb0 + 1) * PC
                ps = ps_sc.tile([PC, 384], f32, tag="sc", name=f"sc{w}")
                nc.tensor.matmul(ps[:, :ncol], kT[:, w * PC:(w + 1) * PC],
                                 qT[:, lb0 * PC:(lb1 + 1) * PC],
                                 start=True, stop=True)
                if mask_eng[w] == "d":
                    nc.vector.scalar_tensor_tensor(
                        mscs[w][:, :ncol],
                        t1b[:, lb0 * PC:(lb1 + 1) * PC],
                        t2s[:, w:w + 1],
                        ps[:, :ncol],
                        ALU.is_ge, ALU.mult)
                else:
                    scb = bpool.tile([PC, 384], f16, name=f"scb{w}")
                    nc.scalar.copy(scb[:, :ncol], ps[:, :ncol])
                    nc.gpsimd.scalar_tensor_tensor(
                        mscs[w][:, :ncol],
                        t1b[:, lb0 * PC:(lb1 + 1) * PC],
                        t2s[:, w:w + 1],
                        scb[:, :ncol],
                        ALU.is_ge, ALU.mult)
            def win_transposes(b):
                # key-major K for csum chunks 15..29 (slots 0..14), 3 per batch
                s0, s1 = 3 * b, min(3 * b + 3, NPRE)
                pst = trA[:, (b % 2) * 3 * D:(b % 2) * 3 * D + 3 * D]
                for i, s in enumerate(range(s0, s1)):
                    nc.tensor.transpose(pst[:, i * D:(i + 1) * D],
                                        kT[:, s * PC:(s + 1) * PC],
                                        ident[:D, :D])
                nc.vector.tensor_copy(ktm[:, s0 * D:s1 * D],
                                      pst[:, :(s1 - s0) * D])
                for s in range(s0, s1):
                    c = 15 + s
                    nc.tensor.matmul(csps[:, 2 * c:2 * c + 2],
                                     ktm[:, s * D:(s + 1) * D],
                                     v16[:, 2 * s:2 * s + 2],
                                     start=True, stop=True)
            def pre_transposes(b):
                # key-major h2 for prefix chunks, 3 per batch; G matmuls
                c0, c1 = 3 * b, min(3 * b + 3, NPRE)
                r = ((b + 1) % 2) * 3 * D
                pst = trA[:, r:r + 3 * D]
                for i, c in enumerate(range(c0, c1)):
                    j = 18 + c
                    nc.tensor.transpose(pst[:, i * D:(i + 1) * D],
                                        hk1[:, j * PC:(j + 1) * PC],
                                        ident[:D, :D])
                nc.vector.tensor_copy(h2t[:, c0 * D:c1 * D],
                                      pst[:, :(c1 - c0) * D])
                for c in range(c0, c1):
                    j = 18 + c
                    nc.tensor.matmul(gps[:, 2 * c:2 * c + 2],
                                     h2t[:, c * D:(c + 1) * D],
                                     v16[:, 2 * j:2 * j + 2],
                                     start=True, stop=True)
            def emit_prefix_csums():
                nc.vector.tensor_copy(gsb[:], gps)
                # prefix csums: csum_c = W3^T G_c + b3 * ssum_c
                for c in range(NPRE):
                    nc.tensor.matmul(csps[:, 2 * c:2 * c + 2],
                                     wsb[:, 5 * D:6 * D],
                                     gsb[:, 2 * c:2 * c + 2],
                                     start=True, stop=False)
                    nc.tensor.matmul(csps[:, 2 * c:2 * c + 2],
                                     b3row[:], ssb[:, 2 * c:2 * c + 2],
                                     start=False, stop=True)
                nc.vector.tensor_copy(csb[:], csps[:, 0:60])
            # ---- explicit emission order (engine queues are in-order, so
            #      emission order is the per-engine schedule)
            srcs_k, dsts_k = [xk, hk0, hk1], [hk0, hk1, kT]
            srcs_q, dsts_q = [xq, hq0, hq1], [hq0, hq1, qT]
            def K(l, i, eng):
                c0, cw = kb[i]
                mlp_block(srcs_k[l], dsts_k[l], 3 + l, c0, cw, l < 2, eng)
            def Q(l, i, eng):
                c0, cw = qb[i]
                mlp_block(srcs_q[l], dsts_q[l], l, c0, cw, l < 2, eng)
            def out_av(lb):
                # window AV terms, accumulated as soon as the masks exist
                oslc = outp[:, 2 * lb:2 * lb + 2]
                for wo in range(3):
                    w = lb + wo
                    lb0 = max(0, w - 2)
                    nc.tensor.matmul(
                        oslc, mscs[w][:, (lb - lb0) * PC:(lb - lb0 + 1) * PC],
                        v16[:, 2 * w:2 * w + 2],
                        start=(wo == 0), stop=False)
            def out_prefix(lb):
                # prefix terms; closes the accumulation group
                oslc = outp[:, 2 * lb:2 * lb + 2]
                qb_ap = qT[:, lb * PC:(lb + 1) * PC]
                for c in range(lb + 15):
                    nc.tensor.matmul(oslc, qb_ap, csb[:, 2 * c:2 * c + 2],
                                     start=False, stop=(c == lb + 14))
                if lb == QCH - 1:
                    nc.vector.tensor_copy(outb[:], outp)
                    nc.sync.dma_start(out_d[:], outb[:])
            # window path first; km/csum batches woven in (batches 5..9 =
            # window chunks 15..29, 0..4 = prefix chunks 0..14)
            K(0, 0, "d"); K(0, 1, "a"); K(0, 2, "d"); Q(0, 0, "a")
            K(0, 3, "d"); K(0, 4, "a"); Q(0, 1, "d"); K(0, 5, "a")
            K(1, 0, "d"); K(1, 1, "a"); Q(0, 2, "d"); K(1, 2, "a")
            nc.tensor.transpose(b3ps, bsb[:, 5:6], ident[:D, :D])
            nc.vector.tensor_copy(b3row[:], b3ps)
            km_batch(5, "d")
            Q(1, 0, "d"); K(1, 3, "a"); Q(0, 3, "d"); K(1, 4, "a")
            km_batch(6, "d")
            for c in range(30):
                j = (c - 15) if c >= 15 else (18 + c)
                nc.tensor.matmul(ssps[:, 2 * c:2 * c + 2], ones128[:],
                                 v16[:, 2 * j:2 * j + 2], start=True, stop=True)
            nc.vector.tensor_copy(ssb[:], ssps)
            Q(1, 1, "d"); K(1, 5, "a"); km_batch(7, "d")
            Q(2, 0, "a"); Q(1, 2, "a")
            K(2, 0, "a")
            score_mask(0); score_mask(1)
            km_batch(8, "d"); csum_batch(5, "d")
            Q(1, 3, "a"); Q(2, 1, "a")
            score_mask(2)
            km_batch(9, "d"); csum_batch(6, "d")
            out_av(0)
            K(2, 1, "a"); Q(2, 2, "a")
            score_mask(3); score_mask(4)
            csum_batch(7, "d")
            out_av(2)
            Q(2, 3, "a"); K(2, 2, "a")
            score_mask(5)
            csum_batch(8, "d")
            out_av(3)
            K(2, 3, "a")
            score_mask(6); score_mask(7)
            csum_batch(9, "d")
            out_av(5)
            K(2, 4, "a")
            score_mask(8)
            out_av(6)
            K(2, 5, "a")
            score_mask(9); score_mask(10)
            out_av(8)
            score_mask(11)
            out_av(9)
            score_mask(12); score_mask(13)
            out_av(11)
            score_mask(14)
            out_av(12)
            score_mask(15); score_mask(16); score_mask(17)
            out_av(15)
            # prefix chunks (Act wb; DVE is busy masking)
            K(0, 6, "a"); K(0, 7, "a"); K(0, 8, "a")
            K(0, 9, "a"); K(0, 10, "a")
            K(1, 6, "a"); km_batch(0, "d")
            K(1, 7, "a"); km_batch(1, "d")
            K(1, 8, "a"); km_batch(2, "d")
            K(1, 9, "a"); km_batch(3, "d")
            K(1, 10, "a"); km_batch(4, "d")
            for c in range(NPRE):
                j = 18 + c
                nc.tensor.matmul(csps[:, 2 * c:2 * c + 2],
                                 km[:, c * D:(c + 1) * D],
                                 v16[:, 2 * j:2 * j + 2],
                                 start=True, stop=False)
                nc.tensor.matmul(csps[:, 2 * c:2 * c + 2],
                                 b3row[:], ssb[:, 2 * c:2 * c + 2],
                                 start=False, stop=True)
            nc.vector.tensor_copy(csb[:], csps[:, 0:60])
            for lb in range(QCH):
                out_prefix(lb)
    nc.compile()
    return nc

def _get_nc():
    if "nc" not in _NC_CACHE:
        _NC_CACHE["nc"] = _build_nc()
    return _NC_CACHE["nc"]

def kernel(x1, x2, x3, x4, wq, bq, wk, bk):
    xs = (x1, x2, x3, x4)
    if not _window_ok(x1, xs):
        return _numpy_fallback(x1, x2, x3, x4, wq, bq, wk, bk)
    in_maps = _shard_host(x1, x2, x3, x4, wq, bq, wk, bk)
    from concourse.bass_utils import run_bass_kernel_spmd
    nc = _get_nc()
    res = run_bass_kernel_spmd(nc, in_maps, list(range(8)))
    return _combine([r["out"] for r in res.results])
